# revision 1
# baseline (speedup 1.0000x reference)
"""BertSelfAttention with gated prompt-prefix branch on 8 Trainium2 cores.

Sharding: data-parallel over batch (B=8 -> 1 batch element per core), no
collectives. Per core, the full attention pipeline runs in a transposed
[feature, seq] layout so that softmax statistics ride through the matmuls:

  qT/kT = W @ hsT          [768, 1024]  (bf16, PE)
  v_aug = hs @ WvT_aug     [1024, 780]  natural layout, 65-col stride per
                           head, col 65h+64 = ones (denominator column)
  scoresT_h = kh @ qh.T    [t, s] via K=64 row-tiled matmuls, 2 heads
                           concurrently on the 128x128 PE array
  expT = exp(SCALE*scoresT + mask[t])   one fused ACT op per tile
  ctxT_aug_h = v_aug_h.T @ expT_h       rows 0..63 ctx, row 64 = sum_t exp
  prefix branch identical with prompt-derived k/v; tanh(gate) folded into
  the prefix v weights on-device
  out_h = ctxT/denom + pctxT/pdenom     (DVE, reciprocal + partition bcast)

Output is produced as outT [768, 1024] fp32 per core; the host transposes
and stacks to [8, 1024, 768].
"""

import numpy as np
import ml_dtypes

import concourse.bass as bass
import concourse.mybir as mybir
import concourse.tile as tile
from concourse.bass_utils import run_bass_kernel_spmd
from concourse.vector_clock import ScopedClock


class SplitDrainTileContext(tile.TileContext):
    """This walrus build rejects >2 sync waits on the kernel-tail Drain
    ("Too many sync wait commands"); split them across SP nops instead."""

    def _drain_and_barrier(self, tick_clock, wait_clock):
        probe = self.nc.sync.nop(nofuse=True, hint="drain_wait_split")
        wait_clock.add_sem_waits(
            probe.ins, ScopedClock({None: tick_clock.global_clock})
        )
        waits = list(probe.ins.sync_info.on_wait or [])
        if len(waits) > 1:
            probe.ins.sync_info.on_wait = waits[:1]
            for i in range(1, len(waits)):
                extra = self.nc.sync.nop(nofuse=True, hint="drain_wait_split")
                extra.ins.sync_info = mybir.SyncInfo(
                    on_wait=waits[i : i + 1], on_update=[]
                )
        drain_inst = self.nc.sync.drain()
        if drain_inst.ins.sync_info is not None:
            drain_inst.ins.sync_info.on_wait = []
        self.nc.all_engine_barrier()
        assert self.sems is not None
        popped = self.nc._tile_sem_poison_stack.pop()
        assert popped is self._sem_poison
        self.nc.clear_and_free_semaphores(list(self.sems.allocated().values()))
        self.nc.all_engine_barrier()

F32 = mybir.dt.float32
BF16 = mybir.dt.bfloat16
AF = mybir.ActivationFunctionType

H, DH, D = 12, 64, 768
S, AT, B = 1024, 64, 8
SCALE = 1.0 / np.sqrt(DH)
NC_D = D // 128  # 6 chunks over feature dim
NC_S = S // 128  # 8 chunks over sequence dim
PAIRS = H // 2  # 6 head pairs
VW = H * (DH + 1)  # 780: v with per-head ones column

_CACHE = {}
LAST_RESULTS = None


def _split_sync_waits(nc, cap=1):
    """Walrus on this image allows very few sync-wait commands per
    instruction (tensor_scalar rejects 2). Hoist excess waits onto
    same-engine nops placed immediately before the instruction."""
    for bb in nc.main_func.blocks:
        cur = list(bb.instructions)
        out = []
        for inst in cur:
            si = inst.sync_info
            waits = list(si.on_wait) if si and si.on_wait else []
            if len(waits) > cap:
                for i in range(0, len(waits) - cap):
                    bi = nc.engines[inst.engine].nop(
                        nofuse=True, hint="wait_split")
                    popped = nc.cur_bb.bb.instructions.pop()
                    assert popped is bi.ins
                    bi.ins.sync_info = mybir.SyncInfo(
                        on_wait=waits[i : i + 1], on_update=[])
                    out.append(bi.ins)
                si.on_wait = waits[len(waits) - cap:]
            out.append(inst)
        bb.instructions[:] = out


def _build_nc():
    nc = bass.Bass()
    hsT = nc.dram_tensor("hsT", [D, S], BF16, kind="ExternalInput")
    wqT = nc.dram_tensor("wqT", [D, D], BF16, kind="ExternalInput")
    wkT = nc.dram_tensor("wkT", [D, D], BF16, kind="ExternalInput")
    wvT = nc.dram_tensor("wvT", [D, VW], BF16, kind="ExternalInput")
    bq = nc.dram_tensor("bq", [D, 1], F32, kind="ExternalInput")
    bk = nc.dram_tensor("bk", [D, 1], F32, kind="ExternalInput")
    bvaug = nc.dram_tensor("bvaug", [128, VW], F32, kind="ExternalInput")
    promptT = nc.dram_tensor("promptT", [D, AT], BF16, kind="ExternalInput")
    mask = nc.dram_tensor("mask", [S, 1], F32, kind="ExternalInput")
    gating = nc.dram_tensor("gating", [128, VW], F32, kind="ExternalInput")
    outT = nc.dram_tensor("outT", [D, S], F32, kind="ExternalOutput")

    with SplitDrainTileContext(nc) as tc:
        _emit(nc, tc, hsT, wqT, wkT, wvT, bq, bk, bvaug, promptT, mask,
              gating, outT)
    _split_sync_waits(nc)
    return nc


def _emit(nc, tc, hsT, wqT, wkT, wvT, bq, bk, bvaug, promptT, mask, gating,
          outT):
    from contextlib import ExitStack

    with ExitStack() as ctx:
        pers = ctx.enter_context(tc.tile_pool(name="pers", bufs=1))

        # ---- SBUF arrays that live into the attention phase ----
        mask_sb = pers.tile([128, NC_S], F32, tag="mask")
        emask_sb = pers.tile([128, NC_S], F32, tag="emask")
        qT_sb = pers.tile([128, NC_D * S], BF16, tag="qT")
        kT_sb = pers.tile([128, NC_D * S], BF16, tag="kT")
        v_sb = pers.tile([128, NC_S * VW], BF16, tag="v")
        pkT_sb = pers.tile([128, NC_D * AT], BF16, tag="pkT")
        pv_sb = pers.tile([128, VW], BF16, tag="pv")

        # ---- projection-phase-only arrays (pool closed afterwards so the
        # attention pools can reuse the space) ----
        proj_cm = tc.tile_pool(name="proj", bufs=1, side="right")
        proj = proj_cm.__enter__()
        hsT_sb = proj.tile([128, NC_D * S], BF16, tag="hsT")
        wqT_sb = proj.tile([128, NC_D * D], BF16, tag="wqT")
        wkT_sb = proj.tile([128, NC_D * D], BF16, tag="wkT")
        wvT_sb = proj.tile([128, NC_D * VW], BF16, tag="wvT")
        pT_sb = proj.tile([128, NC_D * AT], BF16, tag="pT")
        bq_sb = proj.tile([128, NC_D], F32, tag="bq")
        bk_sb = proj.tile([128, NC_D], F32, tag="bk")
        bvaug_sb = proj.tile([128, VW], F32, tag="bvaug")
        graw_sb = proj.tile([128, VW], F32, tag="graw")
        gbc_sb = proj.tile([128, VW], F32, tag="gbc")
        pvtmp_sb = proj.tile([64, VW], F32, tag="pvtmp")

        for src, dst, w in ((wqT, wqT_sb, D), (wkT, wkT_sb, D),
                            (hsT, hsT_sb, S), (wvT, wvT_sb, VW),
                            (promptT, pT_sb, AT)):
            nc.sync.dma_start(
                dst[:].rearrange("p (c s) -> p c s", s=w),
                src[:, :].rearrange("(c p) s -> p c s", p=128))
        # biases / mask: [768,1] & [1024,1] -> [128, nchunks]
        nc.sync.dma_start(bq_sb[:], bq.rearrange("(c p) 1 -> p c", p=128))
        nc.sync.dma_start(bk_sb[:], bk.rearrange("(c p) 1 -> p c", p=128))
        nc.sync.dma_start(mask_sb[:], mask.rearrange("(c p) 1 -> p c", p=128))
        nc.sync.dma_start(bvaug_sb[:], bvaug[:])
        # gating arrives host-replicated to [128, 780] (65 copies per head
        # along the row, broadcast down the partitions)
        nc.sync.dma_start(graw_sb[:], gating[:])
        # tanh, then force the ones-column slots back to 1.0
        nc.scalar.activation(gbc_sb[:], graw_sb[:], AF.Tanh)
        ones_slots = gbc_sb[:, :].rearrange(
            "p (h e) -> p h e", h=H)[:, :, DH:DH + 1]
        nc.vector.memset(ones_slots, 1.0)
        # e^mask, folded into the V rows (incl. ones column) instead of an
        # exp bias: exp(S*x + m_t) == e^{m_t} * exp(S*x), and the ones
        # column then accumulates the correctly-masked denominator.
        nc.scalar.activation(emask_sb[:], mask_sb[:], AF.Exp)

        # SBUF pools that outlive the projection phase — opened before the
        # closeable PSUM pools so the per-side pool stack unwinds LIFO
        exp_pool = ctx.enter_context(tc.tile_pool(name="expp", bufs=4))
        pexp_pool = ctx.enter_context(tc.tile_pool(name="pexpp", bufs=3))

        # ---- PSUM pool for the projection phase (closed afterwards) ----
        mm_cm = tc.tile_pool(name="mm", bufs=2, space="PSUM")
        mm_pool = mm_cm.__enter__()

        # ---- Q/K projections (transposed layout) ----
        for c in range(NC_D):
            for w_sb, b_sb, o_sb in ((wqT_sb, bq_sb, qT_sb),
                                     (wkT_sb, bk_sb, kT_sb)):
                ps = mm_pool.tile([128, S], F32, tag="mm")
                for kc in range(NC_D):
                    lhsT = w_sb[:, kc * D + c * 128: kc * D + (c + 1) * 128]
                    for sb in range(2):
                        nc.tensor.matmul(
                            ps[:, sb * 512:(sb + 1) * 512], lhsT,
                            hsT_sb[:, kc * S + sb * 512: kc * S + (sb + 1) * 512],
                            start=(kc == 0), stop=(kc == NC_D - 1))
                nc.vector.tensor_scalar_add(o_sb[:, c * S:(c + 1) * S],
                                            ps[:], b_sb[:, c:c + 1])

        # PSUM banks 4-7 (on top of mm's 0-3); closed before mm so the
        # pool stack unwinds LIFO, then reopened for pairs 1..5
        sc0_cm = tc.tile_pool(name="scp0", bufs=2, space="PSUM")
        scp = {"p": sc0_cm.__enter__()}

        def prefix_scores(c, pexp):
            sc_pool = scp["p"]
            psp = sc_pool.tile([128, S], F32, tag="sc", name=f"psp_{c}")
            for half in range(2):
                hp = half * 64
                for sb in range(2):
                    nc.tensor.matmul(
                        psp[hp:hp + 64, sb * 512:(sb + 1) * 512],
                        pkT_sb[hp:hp + 64, c * AT:(c + 1) * AT],
                        qT_sb[hp:hp + 64,
                              c * S + sb * 512: c * S + (sb + 1) * 512],
                        tile_position=(hp, hp))
            nc.scalar.activation(pexp[:], psp[:], AF.Exp, scale=SCALE)

        def scores_exp(c, exp_ab, pexp, ctx_mms=None):
            sc_pool = scp["p"]
            """Scores + exp for pair c ([t,s] layout, 2 heads row-tiled);
            optionally interleaves ctx matmuls for chunk tci-1 to keep the
            PE dense."""
            for tci in range(NC_S):
                for half in range(2):
                    hp = half * 64
                    st = sc_pool.tile([128, S], F32, tag="sc",
                                      name=f"st_{c}_{tci}_{half}")
                    lhsT = kT_sb[hp:hp + 64,
                                 c * S + tci * 128: c * S + (tci + 1) * 128]
                    for sb in range(2):
                        nc.tensor.matmul(
                            st[:, sb * 512:(sb + 1) * 512], lhsT,
                            qT_sb[hp:hp + 64,
                                  c * S + sb * 512: c * S + (sb + 1) * 512],
                            tile_position=(hp, 0))
                    nc.scalar.activation(
                        exp_ab[half][:, tci * S:(tci + 1) * S],
                        st[:], AF.Exp, scale=SCALE)
                if tci == 0:
                    if pexp is not None:
                        # prefix scores ride in the first bubble
                        prefix_scores(c, pexp)
                elif ctx_mms is not None:
                    for half in range(2):
                        ctx_mms(half, tci - 1)

        # pair-0 scores start as soon as qT/kT chunk 0 exists, overlapping
        # the V/prompt projections below (ACT would otherwise sit idle).
        # The prefix part waits until pkT exists, in finish_pair(0).
        exp0 = [exp_pool.tile([128, NC_S * S], BF16, tag="exp",
                              name=f"exp_0_{i}") for i in range(2)]
        pexp0 = pexp_pool.tile([128, S], BF16, tag="pexp", name="pexp0")
        scores_exp(0, exp0, None)

        # ---- V projection (natural layout, augmented ones column) ----
        for sc in range(NC_S):
            ps = mm_pool.tile([128, S], F32, tag="mm")
            for kc in range(NC_D):
                lhsT = hsT_sb[:, kc * S + sc * 128: kc * S + (sc + 1) * 128]
                nc.tensor.matmul(ps[:, 0:512], lhsT,
                                 wvT_sb[:, kc * VW: kc * VW + 512],
                                 start=(kc == 0), stop=(kc == NC_D - 1))
                nc.tensor.matmul(ps[:, 512:VW], lhsT,
                                 wvT_sb[:, kc * VW + 512: (kc + 1) * VW],
                                 start=(kc == 0), stop=(kc == NC_D - 1))
            vt = proj.tile([128, VW], F32, tag="vtmp", name=f"vt{sc}",
                           bufs=2)
            nc.vector.tensor_add(vt[:], ps[:, 0:VW], bvaug_sb[:])
            nc.vector.tensor_scalar_mul(v_sb[:, sc * VW:(sc + 1) * VW],
                                        vt[:], emask_sb[:, sc:sc + 1])

        # ---- prompt K projection (transposed) ----
        for c in range(NC_D):
            ps = mm_pool.tile([128, S], F32, tag="mm")
            for kc in range(NC_D):
                nc.tensor.matmul(
                    ps[:, 0:AT],
                    wkT_sb[:, kc * D + c * 128: kc * D + (c + 1) * 128],
                    pT_sb[:, kc * AT:(kc + 1) * AT],
                    start=(kc == 0), stop=(kc == NC_D - 1))
            nc.vector.tensor_scalar_add(pkT_sb[:, c * AT:(c + 1) * AT],
                                        ps[:, 0:AT], bk_sb[:, c:c + 1])

        # ---- prompt V projection (natural, gate-scaled, duplicated) ----
        ps = mm_pool.tile([128, S], F32, tag="mm")
        for kc in range(NC_D):
            lhsT = pT_sb[:, kc * AT:(kc + 1) * AT]
            nc.tensor.matmul(ps[0:AT, 0:512], lhsT,
                             wvT_sb[:, kc * VW: kc * VW + 512],
                             start=(kc == 0), stop=(kc == NC_D - 1))
            nc.tensor.matmul(ps[0:AT, 512:VW], lhsT,
                             wvT_sb[:, kc * VW + 512: (kc + 1) * VW],
                             start=(kc == 0), stop=(kc == NC_D - 1))
        nc.vector.tensor_add(pvtmp_sb[:], ps[0:AT, 0:VW], bvaug_sb[0:AT, :])
        nc.vector.tensor_mul(pv_sb[0:AT, :], pvtmp_sb[:], gbc_sb[0:AT, :])
        nc.sync.dma_start(pv_sb[AT:128, :], pv_sb[0:AT, :])

        sc0_cm.__exit__(None, None, None)
        proj_cm.__exit__(None, None, None)
        mm_cm.__exit__(None, None, None)

        # ---- remaining attention pools (reuse the projection PSUM) ----
        scp["p"] = ctx.enter_context(
            tc.tile_pool(name="scp", bufs=2, space="PSUM"))
        ctx_pool = ctx.enter_context(
            tc.tile_pool(name="ctxp", bufs=2, space="PSUM"))
        norm_pool = ctx.enter_context(tc.tile_pool(name="normp", bufs=2))
        out_pool = ctx.enter_context(tc.tile_pool(name="outp", bufs=2))
        dscr_pool = ctx.enter_context(
            tc.tile_pool(name="dscr", bufs=2, space="DRAM"))

        def make_ctx_mms(c, cps_ab, exp_ab):
            def ctx_mms(half, tci):
                h = 2 * c + half
                lhsT = v_sb[:, tci * VW + h * 65: tci * VW + h * 65 + 65]
                for sb in range(2):
                    nc.tensor.matmul(
                        cps_ab[half][:, sb * 512:(sb + 1) * 512], lhsT,
                        exp_ab[half][:, tci * S + sb * 512:
                                     tci * S + (sb + 1) * 512],
                        start=(tci == 0), stop=(tci == NC_S - 1))
            return ctx_mms

        def finish_stage1(c, exp_ab, pexp, cps_ab):
            """Prefix ctx matmuls, psum evacuation (frees banks fast), and
            the denominator reciprocal/broadcast chain. The slow combine
            (waits on the broadcast DMA) is deferred to finish_stage2 so it
            never blocks the DVE FIFO ahead of psum-freeing copies."""
            state = []
            for half in range(2):
                h = 2 * c + half
                hp = half * 64
                cps = cps_ab[half]
                pps = scp["p"].tile([128, S], F32, tag="sc",
                                    name=f"pps_{c}_{half}")
                for sb in range(2):
                    nc.tensor.matmul(
                        pps[0:65, sb * 512:(sb + 1) * 512],
                        pv_sb[hp:hp + 64, h * 65: h * 65 + 65],
                        pexp[hp:hp + 64, sb * 512:(sb + 1) * 512],
                        tile_position=(hp, 0))

                ce = norm_pool.tile([65, S], F32, tag="ce", bufs=4,
                                    name=f"ce_{c}_{half}")
                pe_ev = norm_pool.tile([65, S], F32, tag="pe_ev", bufs=4,
                                       name=f"pe_{c}_{half}")
                nc.vector.tensor_copy(ce[:], cps[:])
                nc.vector.tensor_copy(pe_ev[:], pps[0:65, :])
                # denominator rows -> DMA-reshape across partitions ->
                # cheap wide reciprocal -> DRAM -> broadcast
                dresh = norm_pool.tile([128, 16], F32, tag="dresh", bufs=4,
                                       name=f"dr_{c}_{half}")
                nc.sync.dma_start(dresh[:, 0:8], ce[64:65, :])
                nc.sync.dma_start(dresh[:, 8:16], pe_ev[64:65, :])
                rrec = norm_pool.tile([128, 16], F32, tag="rrec", bufs=4,
                                      name=f"rr_{c}_{half}")
                nc.vector.reciprocal(rrec[:], dresh[:])
                r_d = dscr_pool.tile([1, 2 * S], F32, tag="rd", bufs=4,
                                     name=f"rd_{c}_{half}")
                nc.sync.dma_start(r_d[0:1, 0:S], rrec[:, 0:8])
                nc.sync.dma_start(r_d[0:1, S:2 * S], rrec[:, 8:16])
                r_bc = norm_pool.tile([64, 2 * S], F32, tag="rbc", bufs=4,
                                      name=f"rbc_{c}_{half}")
                r_src = bass.AP(r_d[:].tensor, r_d[:].offset,
                                [[0, 64], [1, 2 * S]])
                nc.sync.dma_start(r_bc[:], r_src)
                state.append((h, ce, pe_ev, r_bc))
            return state

        def finish_stage2(c, state):
            for h, ce, pe_ev, r_bc in state:
                # normalize in place, combine on GpSimd
                nc.vector.tensor_mul(ce[0:64, :], ce[0:64, :], r_bc[:, 0:S])
                nc.vector.tensor_mul(pe_ev[0:64, :], pe_ev[0:64, :],
                                     r_bc[:, S:2 * S])
                ot = out_pool.tile([64, S], F32, tag="ot",
                                   name=f"ot_{c}_{h}")
                nc.gpsimd.tensor_add(ot[:], ce[0:64, :], pe_ev[0:64, :])
                nc.sync.dma_start(outT[h * 64:(h + 1) * 64, :], ot[:])

        # pair 0: ctx for the pre-computed exps, then the remaining pairs
        # with ctx interleaved behind their own score/exp stream
        cps0 = [ctx_pool.tile([65, S], F32, tag="ctx", name=f"cps_0_{i}")
                for i in range(2)]
        ctx0 = make_ctx_mms(0, cps0, exp0)
        prefix_scores(0, pexp0)
        for tci in range(NC_S):
            for half in range(2):
                ctx0(half, tci)
        pending = (0, finish_stage1(0, exp0, pexp0, cps0))

        for c in range(1, PAIRS):
            exp_ab = [exp_pool.tile([128, NC_S * S], BF16, tag="exp",
                                    name=f"exp_{c}_{i}")
                      for i in range(2)]
            pexp = pexp_pool.tile([128, S], BF16, tag="pexp",
                                  name=f"pexp_{c}")
            cps_ab = [ctx_pool.tile([65, S], F32, tag="ctx",
                                    name=f"cps_{c}_{i}")
                      for i in range(2)]
            cmm = make_ctx_mms(c, cps_ab, exp_ab)
            scores_exp(c, exp_ab, pexp, ctx_mms=cmm)
            finish_stage2(*pending)
            for half in range(2):
                cmm(half, NC_S - 1)
            pending = (c, finish_stage1(c, exp_ab, pexp, cps_ab))
        finish_stage2(*pending)


def _prep_inputs(hidden_states, prompt_tokens, gating_factor, attention_mask,
                 Wq, bq, Wk, bk, Wv, bv):
    bf = ml_dtypes.bfloat16
    hs = np.asarray(hidden_states, np.float32)
    mask = np.asarray(attention_mask, np.float32).reshape(B, S)
    wqT = np.ascontiguousarray(np.asarray(Wq, np.float32).T).astype(bf)
    wkT = np.ascontiguousarray(np.asarray(Wk, np.float32).T).astype(bf)
    # augmented WvT: [din, 780], col 65h+j = Wv.T[:, 64h+j], col 65h+64 = 0
    wvT_f = np.asarray(Wv, np.float32).T  # [din, dout]
    wvT_aug = np.zeros((D, VW), np.float32)
    idx = np.arange(D)
    aug_cols = (idx // DH) * (DH + 1) + (idx % DH)
    wvT_aug[:, aug_cols] = wvT_f
    wvT_aug = wvT_aug.astype(bf)
    bq_c = np.asarray(bq, np.float32).reshape(D, 1)
    bk_c = np.asarray(bk, np.float32).reshape(D, 1)
    bv_aug = np.zeros(VW, np.float32)
    bv_aug[aug_cols] = np.asarray(bv, np.float32)
    bv_aug[DH::DH + 1] = 1.0
    bvaug_bc = np.ascontiguousarray(
        np.broadcast_to(bv_aug, (128, VW)), np.float32)
    pT = np.ascontiguousarray(
        np.asarray(prompt_tokens, np.float32)[0].T).astype(bf)
    gat_row = np.repeat(
        np.asarray(gating_factor, np.float32).reshape(H), DH + 1)
    gat = np.ascontiguousarray(
        np.broadcast_to(gat_row, (128, VW)), np.float32)

    shared = dict(wqT=wqT, wkT=wkT, wvT=wvT_aug, bq=bq_c, bk=bk_c,
                  bvaug=bvaug_bc, promptT=pT, gating=gat)
    in_maps = []
    for b in range(B):
        m = dict(shared)
        m["hsT"] = np.ascontiguousarray(hs[b].T).astype(bf)
        m["mask"] = np.ascontiguousarray(mask[b].reshape(S, 1))
        in_maps.append(m)
    return in_maps


def kernel(**inputs):
    global LAST_RESULTS
    if "nc" not in _CACHE:
        _CACHE["nc"] = _build_nc()
    nc = _CACHE["nc"]
    in_maps = _prep_inputs(**inputs)
    res = None
    for attempt in range(3):
        try:
            res = run_bass_kernel_spmd(nc, in_maps, list(range(B)))
            break
        except ModuleNotFoundError:
            # BASS_TRACE set but this image lacks antenv.axon_hooks
            import os

            os.environ["BASS_NEVER_TRACE"] = "1"
            if attempt == 2:
                raise
        except Exception:
            # transient NRT_EXEC_UNIT_UNRECOVERABLE on a cold device has
            # been observed; a retry on the same session recovers
            if attempt == 2:
                raise
    LAST_RESULTS = res
    out = np.empty((B, S, D), np.float32)
    for b in range(B):
        out[b] = res.results[b]["outT"].T
    return out



# revision 9
# speedup vs baseline: 1.2508x; 1.2508x over previous
"""BertSelfAttention with gated prompt-prefix branch on 8 Trainium2 cores.

Sharding: data-parallel over batch (B=8 -> 1 batch element per core), no
collectives. Per core the kernel is organized as a software pipeline whose
clock is the ScalarE (ACT) exp spine: 6 head-pairs x 17 exp tiles of
[128, 1024] each (~1.15us per ACT). All other engines are scheduled so the
PE never idles (HAM stays warm) and the ACT never waits:

  window c (one head pair, 16 score beats + prefix):
    PE : score MMs (pair c)            2 MMs/beat, h0+h1 row-concurrent
         ctx MMs (pair c-1)            accumulate [65,1024] psum, K=128
         prefix score/ctx MMs          row-concurrent halves
         projection filler             Q/K/V/prompt chunks woven in gaps
    ACT: exp of score tile (beat b)    the spine, ~100% busy
    DVE: psum evacuations (frees the 2-slot score pool + ctx accumulators),
         projection bias adds, softmax normalization muls
    GpS: final combine adds
    DMA: input staging, denominator reciprocal broadcast, output rows

Scores for a beat pack both heads: st[:, 0:512] = h0, st[:, 512:1024] = h1
(one s-half each) so the two K=64 matmuls occupy disjoint PE row halves and
run concurrently; one ACT covers both. Softmax denominators ride an extra
ones-column in the augmented V (col 65h+64), giving [65,1024] ctx tiles
whose row 64 is sum_t exp. exp(mask) is folded into V rows.

Output is produced as outT [768, 1024] fp32 per core; the host transposes
and stacks to [8, 1024, 768].
"""

import numpy as np
import ml_dtypes

import concourse.bass as bass
import concourse.mybir as mybir
import concourse.tile as tile
from concourse.bass_utils import run_bass_kernel_spmd
from concourse.vector_clock import ScopedClock


class SplitDrainTileContext(tile.TileContext):
    """This walrus build rejects >2 sync waits on the kernel-tail Drain
    ("Too many sync wait commands"); split them across SP nops instead."""

    def _drain_and_barrier(self, tick_clock, wait_clock):
        probe = self.nc.sync.nop(nofuse=True, hint="drain_wait_split")
        wait_clock.add_sem_waits(
            probe.ins, ScopedClock({None: tick_clock.global_clock})
        )
        waits = list(probe.ins.sync_info.on_wait or [])
        if len(waits) > 1:
            probe.ins.sync_info.on_wait = waits[:1]
            for i in range(1, len(waits)):
                extra = self.nc.sync.nop(nofuse=True, hint="drain_wait_split")
                extra.ins.sync_info = mybir.SyncInfo(
                    on_wait=waits[i : i + 1], on_update=[]
                )
        drain_inst = self.nc.sync.drain()
        if drain_inst.ins.sync_info is not None:
            drain_inst.ins.sync_info.on_wait = []
        self.nc.all_engine_barrier()
        assert self.sems is not None
        popped = self.nc._tile_sem_poison_stack.pop()
        assert popped is self._sem_poison
        self.nc.clear_and_free_semaphores(list(self.sems.allocated().values()))
        self.nc.all_engine_barrier()

F32 = mybir.dt.float32
BF16 = mybir.dt.bfloat16
AF = mybir.ActivationFunctionType

H, DH, D = 12, 64, 768
S, AT, B = 1024, 64, 8
SCALE = 1.0 / np.sqrt(DH)
NC_D = D // 128  # 6 chunks over feature dim
NC_S = S // 128  # 8 chunks over sequence dim
PAIRS = H // 2  # 6 head pairs
VW = H * (DH + 1)  # 780: v with per-head ones column

_CACHE = {}
LAST_RESULTS = None


def _split_sync_waits(nc, cap=1):
    """Walrus on this image allows very few sync-wait commands per
    instruction (tensor_scalar rejects 2). Hoist excess waits onto
    same-engine nops placed immediately before the instruction."""
    for bb in nc.main_func.blocks:
        cur = list(bb.instructions)
        out = []
        for inst in cur:
            si = inst.sync_info
            waits = list(si.on_wait) if si and si.on_wait else []
            if len(waits) > cap:
                for i in range(0, len(waits) - cap):
                    bi = nc.engines[inst.engine].nop(
                        nofuse=True, hint="wait_split")
                    popped = nc.cur_bb.bb.instructions.pop()
                    assert popped is bi.ins
                    bi.ins.sync_info = mybir.SyncInfo(
                        on_wait=waits[i : i + 1], on_update=[])
                    out.append(bi.ins)
                si.on_wait = waits[len(waits) - cap:]
            out.append(inst)
        bb.instructions[:] = out


def _build_nc():
    nc = bass.Bass()
    hsT = nc.dram_tensor("hsT", [D, S], BF16, kind="ExternalInput")
    wqT = nc.dram_tensor("wqT", [D, D], BF16, kind="ExternalInput")
    wkT = nc.dram_tensor("wkT", [D, D], BF16, kind="ExternalInput")
    wvT = nc.dram_tensor("wvT", [D, VW], BF16, kind="ExternalInput")
    bq = nc.dram_tensor("bq", [D, 1], F32, kind="ExternalInput")
    bk = nc.dram_tensor("bk", [D, 1], F32, kind="ExternalInput")
    bvaug = nc.dram_tensor("bvaug", [128, VW], F32, kind="ExternalInput")
    promptT = nc.dram_tensor("promptT", [D, AT], BF16, kind="ExternalInput")
    mask = nc.dram_tensor("mask", [S, 1], F32, kind="ExternalInput")
    gating = nc.dram_tensor("gating", [128, VW], F32, kind="ExternalInput")
    outT = nc.dram_tensor("outT", [D, S], F32, kind="ExternalOutput")

    with SplitDrainTileContext(nc) as tc:
        _emit(nc, tc, hsT, wqT, wkT, wvT, bq, bk, bvaug, promptT, mask,
              gating, outT)
    _split_sync_waits(nc)
    return nc


def _emit(nc, tc, hsT, wqT, wkT, wvT, bq, bk, bvaug, promptT, mask, gating,
          outT):
    from contextlib import ExitStack

    with ExitStack() as ctx:
        pers = ctx.enter_context(tc.tile_pool(name="pers", bufs=1))

        # ---- SBUF persistent arrays ----
        mask_sb = pers.tile([128, NC_S], F32, tag="mask")
        emask_sb = pers.tile([128, NC_S], F32, tag="emask")
        qT_sb = pers.tile([128, NC_D * S], BF16, tag="qT")
        kT_sb = pers.tile([128, NC_D * S], BF16, tag="kT")
        v_sb = pers.tile([128, NC_S * VW], BF16, tag="v")
        pkT_sb = pers.tile([128, NC_D * AT], BF16, tag="pkT")
        pv_sb = pers.tile([128, VW], BF16, tag="pv")
        hsT_sb = pers.tile([128, NC_D * S], BF16, tag="hsT")
        wqT_sb = pers.tile([128, NC_D * D], BF16, tag="wqT")
        wkT_sb = pers.tile([128, NC_D * D], BF16, tag="wkT")
        wvT_sb = pers.tile([128, NC_D * VW], BF16, tag="wvT")
        pT_sb = pers.tile([128, NC_D * AT], BF16, tag="pT")
        bq_sb = pers.tile([128, NC_D], F32, tag="bq")
        bk_sb = pers.tile([128, NC_D], F32, tag="bk")
        bvaug_sb = pers.tile([128, VW], F32, tag="bvaug")
        gbc_sb = pers.tile([128, VW], F32, tag="gbc")
        pvtmp_sb = pers.tile([64, VW], F32, tag="pvtmp")

        # ---- SBUF working pools ----
        exp_pool = ctx.enter_context(tc.tile_pool(name="expp", bufs=20))
        pexp_pool = ctx.enter_context(tc.tile_pool(name="pexpp", bufs=2))
        vt_pool = ctx.enter_context(tc.tile_pool(name="vtp", bufs=2))
        ce_pool = ctx.enter_context(tc.tile_pool(name="cep", bufs=4))
        pe_pool = ctx.enter_context(tc.tile_pool(name="pep", bufs=4))
        nrm_pool = ctx.enter_context(tc.tile_pool(name="nrmp", bufs=4))
        rbc_pool = ctx.enter_context(tc.tile_pool(name="rbcp", bufs=2))
        ot_pool = ctx.enter_context(tc.tile_pool(name="otp", bufs=2))
        dscr_pool = ctx.enter_context(
            tc.tile_pool(name="dscr", bufs=4, space="DRAM"))

        # ---- PSUM pools: 2-slot general (4 banks) + ctx accums (4 banks)
        ps_pool = ctx.enter_context(
            tc.tile_pool(name="psp", bufs=2, space="PSUM"))
        ctx_pool = ctx.enter_context(
            tc.tile_pool(name="ctxp", bufs=2, space="PSUM"))

        # ---- input DMAs, critical-path first ----
        def dma_w_slice(dst_sb, src, c0, c1):
            dst = dst_sb[:].rearrange("p (k j) -> p k j", j=D)[
                :, :, c0 * 128 : c1 * 128]
            src_ap = src[:, c0 * 128 : c1 * 128].rearrange(
                "(k p) j -> p k j", p=128)
            nc.sync.dma_start(dst, src_ap)

        dma_w_slice(wkT_sb, wkT, 0, 1)
        dma_w_slice(wqT_sb, wqT, 0, 1)
        nc.sync.dma_start(
            hsT_sb[:].rearrange("p (c s) -> p c s", s=S),
            hsT[:, :].rearrange("(c p) s -> p c s", p=128))
        nc.sync.dma_start(bq_sb[:], bq.rearrange("(c p) 1 -> p c", p=128))
        nc.sync.dma_start(bk_sb[:], bk.rearrange("(c p) 1 -> p c", p=128))
        nc.sync.dma_start(mask_sb[:], mask.rearrange("(c p) 1 -> p c", p=128))
        nc.sync.dma_start(gbc_sb[:], gating[:])
        nc.sync.dma_start(bvaug_sb[:], bvaug[:])
        dma_w_slice(wkT_sb, wkT, 1, NC_D)
        dma_w_slice(wqT_sb, wqT, 1, NC_D)
        nc.sync.dma_start(
            wvT_sb[:].rearrange("p (c s) -> p c s", s=VW),
            wvT[:, :].rearrange("(c p) s -> p c s", p=128))
        nc.sync.dma_start(
            pT_sb[:].rearrange("p (c s) -> p c s", s=AT),
            promptT[:, :].rearrange("(c p) s -> p c s", p=128))

        # warmup ACTs (loads the exp/tanh table set early, off the spine)
        nc.scalar.activation(gbc_sb[:], gbc_sb[:], AF.Tanh)
        ones_slots = gbc_sb[:, :].rearrange(
            "p (h e) -> p h e", h=H)[:, :, DH:DH + 1]
        nc.vector.memset(ones_slots, 1.0)
        nc.scalar.activation(emask_sb[:], mask_sb[:], AF.Exp)

        # ---------------- unit generators ----------------
        def proj_qk(w_sb, b_sb, o_sb, c, half):
            ps = ps_pool.tile([128, S], F32, tag="ps",
                              name=f"qk_{id(w_sb)}_{c}_{half}")
            for kc in range(NC_D):
                nc.tensor.matmul(
                    ps[:, 0:512],
                    w_sb[:, kc * D + c * 128 : kc * D + (c + 1) * 128],
                    hsT_sb[:, kc * S + half * 512 : kc * S + half * 512 + 512],
                    start=(kc == 0), stop=(kc == NC_D - 1))
            nc.vector.tensor_scalar_add(
                o_sb[:, c * S + half * 512 : c * S + half * 512 + 512],
                ps[:, 0:512], b_sb[:, c:c + 1])

        def proj_v(sc, half):
            off, w = (0, 512) if half == 0 else (512, VW - 512)
            ps = ps_pool.tile([128, S], F32, tag="ps", name=f"v_{sc}_{half}")
            for kc in range(NC_D):
                nc.tensor.matmul(
                    ps[:, 0:w],
                    hsT_sb[:, kc * S + sc * 128 : kc * S + (sc + 1) * 128],
                    wvT_sb[:, kc * VW + off : kc * VW + off + w],
                    start=(kc == 0), stop=(kc == NC_D - 1))
            vt = vt_pool.tile([128, 512], F32, tag="vt",
                              name=f"vt_{sc}_{half}")
            nc.vector.tensor_add(vt[:, 0:w], ps[:, 0:w],
                                 bvaug_sb[:, off:off + w])
            nc.vector.tensor_scalar_mul(
                v_sb[:, sc * VW + off : sc * VW + off + w],
                vt[:, 0:w], emask_sb[:, sc:sc + 1])

        def prompt_k(grp):
            cs = range(3 * grp, 3 * grp + 3)
            ps = ps_pool.tile([128, S], F32, tag="ps", name=f"pk_{grp}")
            for i, c in enumerate(cs):
                for kc in range(NC_D):
                    nc.tensor.matmul(
                        ps[:, i * AT : (i + 1) * AT],
                        wkT_sb[:, kc * D + c * 128 : kc * D + (c + 1) * 128],
                        pT_sb[:, kc * AT : (kc + 1) * AT],
                        start=(kc == 0), stop=(kc == NC_D - 1))
            for i, c in enumerate(cs):
                nc.vector.tensor_scalar_add(
                    pkT_sb[:, c * AT : (c + 1) * AT],
                    ps[:, i * AT : (i + 1) * AT], bk_sb[:, c:c + 1])

        def prompt_v(half):
            off, w = (0, 512) if half == 0 else (512, VW - 512)
            ps = ps_pool.tile([128, S], F32, tag="ps", name=f"pv_{half}")
            for kc in range(NC_D):
                nc.tensor.matmul(
                    ps[0:AT, 0:w],
                    pT_sb[:, kc * AT : (kc + 1) * AT],
                    wvT_sb[:, kc * VW + off : kc * VW + off + w],
                    start=(kc == 0), stop=(kc == NC_D - 1))
            nc.vector.tensor_add(pvtmp_sb[:, off:off + w], ps[0:AT, 0:w],
                                 bvaug_sb[0:AT, off:off + w])
            nc.vector.tensor_mul(pv_sb[0:AT, off:off + w],
                                 pvtmp_sb[:, off:off + w],
                                 gbc_sb[0:AT, off:off + w])

        def pv_mirror():
            nc.sync.dma_start(pv_sb[AT:128, :], pv_sb[0:AT, :])

        # scores: one beat = (pair c, tci, sb); tile holds [h0 512 | h1 512]
        def score_beat(c, tci, sb, exps):
            st = ps_pool.tile([128, S], F32, tag="ps",
                              name=f"st_{c}_{tci}_{sb}")
            for h in range(2):
                hp = h * 64
                nc.tensor.matmul(
                    st[:, h * 512 : h * 512 + 512],
                    kT_sb[hp:hp + 64, c * S + tci * 128 : c * S + (tci + 1) * 128],
                    qT_sb[hp:hp + 64, c * S + sb * 512 : c * S + sb * 512 + 512])
            ex = exp_pool.tile([128, S], BF16, tag="exp",
                               name=f"exp_{c}_{tci}_{sb}")
            nc.scalar.activation(ex[:], st[:], AF.Exp, scale=SCALE)
            exps[(tci, sb)] = ex

        # ctx accumulation for (pair c, head-half h) over one tci
        def ctx_unit(c, h, tci, exps, cps):
            lhsT = v_sb[:, tci * VW + (2 * c + h) * 65 :
                        tci * VW + (2 * c + h) * 65 + 65]
            for sb in range(2):
                nc.tensor.matmul(
                    cps[h][:, sb * 512 : (sb + 1) * 512], lhsT,
                    exps[(tci, sb)][:, h * 512 : h * 512 + 512],
                    start=(tci == 0), stop=(tci == NC_S - 1))

        def psp_unit(c):
            psp = ps_pool.tile([128, S], F32, tag="ps", name=f"psp_{c}")
            for sb in range(2):
                for h in range(2):
                    hp = h * 64
                    nc.tensor.matmul(
                        psp[hp:hp + 64, sb * 512 : (sb + 1) * 512],
                        pkT_sb[hp:hp + 64, c * AT : (c + 1) * AT],
                        qT_sb[hp:hp + 64,
                              c * S + sb * 512 : c * S + sb * 512 + 512],
                        tile_position=(hp, hp))
            pexp = pexp_pool.tile([128, S], BF16, tag="pexp",
                                  name=f"pexp_{c}")
            nc.scalar.activation(pexp[:], psp[:], AF.Exp, scale=SCALE)
            return pexp

        def pps_unit(c, h, pexp, pes):
            hp = h * 64
            pps = ps_pool.tile([128, S], F32, tag="ps", name=f"pps_{c}_{h}")
            for sb in range(2):
                nc.tensor.matmul(
                    pps[0:65, sb * 512 : (sb + 1) * 512],
                    pv_sb[hp:hp + 64, (2 * c + h) * 65 : (2 * c + h) * 65 + 65],
                    pexp[hp:hp + 64, sb * 512 : (sb + 1) * 512],
                    tile_position=(hp, 0))
            pe = pe_pool.tile([65, S], F32, tag="pe", name=f"pe_{c}_{h}")
            nc.vector.tensor_copy(pe[:], pps[0:65, :])
            pes[h] = pe

        def stage1a(c, cps):
            """PSUM evacuation of the ctx accumulators — emitted first at a
            window boundary so the ctx pool frees before ctx MMs queue."""
            ces = []
            for h in range(2):
                ce = ce_pool.tile([65, S], F32, tag="ce", name=f"ce_{c}_{h}")
                nc.vector.tensor_copy(ce[:], cps[h][:])
                ces.append(ce)
            return ces

        def stage1b(c, ces, pes):
            """Denominator reciprocal + partition-broadcast via DRAM."""
            state = []
            for h in range(2):
                ce = ces[h]
                dresh = nrm_pool.tile([128, 16], F32, tag="dresh",
                                      name=f"dr_{c}_{h}")
                nc.sync.dma_start(dresh[:, 0:8], ce[64:65, :])
                nc.sync.dma_start(dresh[:, 8:16], pes[h][64:65, :])
                rrec = nrm_pool.tile([128, 16], F32, tag="rrec",
                                     name=f"rr_{c}_{h}")
                nc.vector.reciprocal(rrec[:], dresh[:])
                r_d = dscr_pool.tile([1, 2 * S], F32, tag="rd",
                                     name=f"rd_{c}_{h}")
                nc.sync.dma_start(r_d[0:1, 0:S], rrec[:, 0:8])
                nc.sync.dma_start(r_d[0:1, S:2 * S], rrec[:, 8:16])
                r_bc = rbc_pool.tile([64, 2 * S], F32, tag="rbc",
                                     name=f"rbc_{c}_{h}")
                r_src = bass.AP(r_d[:].tensor, r_d[:].offset,
                                [[0, 64], [1, 2 * S]])
                nc.sync.dma_start(r_bc[:], r_src)
                state.append((h, ce, pes[h], r_bc))
            return state

        def stage2(c, state):
            for h, ce, pe, r_bc in state:
                nc.vector.tensor_mul(ce[0:64, :], ce[0:64, :], r_bc[:, 0:S])
                nc.vector.tensor_mul(pe[0:64, :], pe[0:64, :],
                                     r_bc[:, S:2 * S])
                ot = ot_pool.tile([64, S], F32, tag="ot",
                                  name=f"ot_{c}_{h}")
                nc.gpsimd.tensor_add(ot[:], ce[0:64, :], pe[0:64, :])
                nc.sync.dma_start(
                    outT[(2 * c + h) * 64 : (2 * c + h) * 64 + 64, :], ot[:])

        # ---------------- schedule ----------------
        # startup projections (first Q/K chunks for pair-0 scores)
        for half in range(2):
            proj_qk(wkT_sb, bk_sb, kT_sb, 0, half)
        for half in range(2):
            proj_qk(wqT_sb, bq_sb, qT_sb, 0, half)

        # per-window projection-filler unit lists
        def qk_units(c):
            return [lambda c=c, h=h: proj_qk(wkT_sb, bk_sb, kT_sb, c, h)
                    for h in range(2)] + \
                   [lambda c=c, h=h: proj_qk(wqT_sb, bq_sb, qT_sb, c, h)
                    for h in range(2)]

        proj_sched = {
            0: [lambda: proj_v(0, 0), lambda: proj_v(0, 1),
                lambda: proj_v(1, 0), lambda: proj_v(1, 1),
                lambda: prompt_k(0), lambda: prompt_k(1),
                lambda: proj_v(2, 0), lambda: proj_v(2, 1),
                lambda: prompt_v(0), lambda: prompt_v(1), pv_mirror]
               + qk_units(1),
            1: [lambda sc=sc, h=h: proj_v(sc, h)
                for sc in range(3, 8) for h in range(2)] + qk_units(2),
            2: qk_units(3),
            3: qk_units(4),
            4: qk_units(5),
            5: [],
        }

        exps_prev = None
        pes_by_pair = {}
        pending_s1 = None  # (c, cps, pes) awaiting stage1 at next window
        pending_s2 = None  # (c, state) awaiting stage2
        pexp_prev = None

        for c in range(PAIRS):
            exps = {}
            projq = list(proj_sched[c])
            ctxq = []
            if c >= 1:
                cps_cur = [ctx_pool.tile([65, S], F32, tag="ctx",
                                         name=f"cps_{c - 1}_{h}")
                           for h in range(2)]
                ctxq = [(h, tci) for tci in range(NC_S) for h in range(2)]
            else:
                cps_cur = None

            # interleave: ctx unit then proj unit per gap
            def pop_units(n_ctx, n_proj):
                for _ in range(n_ctx):
                    if ctxq:
                        h, tci = ctxq.pop(0)
                        ctx_unit(c - 1, h, tci, exps_prev, cps_cur)
                for _ in range(n_proj):
                    if projq:
                        projq.pop(0)()

            s1_ces = None
            for b in range(16):
                tci, sb = b // 2, b % 2
                score_beat(c, tci, sb, exps)
                if b == 0:
                    # DVE order at window boundary: ctx-psum evac first
                    # (unblocks ctx MMs), then prefix-psum evac (unblocks
                    # the score-slot rotation), then the recip chain.
                    if pending_s1 is not None:
                        s1_ces = stage1a(pending_s1[0], pending_s1[1])
                    if c >= 1:
                        pes_by_pair[c - 1] = {}
                        pps_unit(c - 1, 0, pexp_prev, pes_by_pair[c - 1])
                    else:
                        pop_units(0, 1)
                elif b == 1:
                    if c >= 1:
                        pps_unit(c - 1, 1, pexp_prev, pes_by_pair[c - 1])
                    else:
                        pop_units(0, 1)
                elif b == 2:
                    if pending_s1 is not None:
                        pc = pending_s1[0]
                        pending_s2 = (pc, stage1b(pc, s1_ces,
                                                  pending_s1[2]))
                        pending_s1 = None
                    pop_units(1, 1)
                else:
                    # distribute ctx + proj units over gaps 3..15
                    pop_units(1 if b % 2 == 0 else 2, 1)
                    if b == 8 and pending_s2 is not None:
                        stage2(*pending_s2)
                        pending_s2 = None
            # drain leftovers
            while ctxq or projq:
                pop_units(2, 1)
            pexp_cur = psp_unit(c)

            if c >= 1:
                pending_s1 = (c - 1, cps_cur, pes_by_pair[c - 1])
            exps_prev = exps
            pexp_prev = pexp_cur

        # ---------------- tail: pair 5 ----------------
        cps_last = [ctx_pool.tile([65, S], F32, tag="ctx",
                                  name=f"cps_5_{h}")
                    for h in range(2)]
        # stage1(4) first to free ctx slots fast
        pc, pcps, ppes = pending_s1
        s2_4 = (pc, stage1b(pc, stage1a(pc, pcps), ppes))
        pending_s1 = None
        for tci in range(NC_S):
            for h in range(2):
                ctx_unit(5, h, tci, exps_prev, cps_last)
            if tci == 3 and s2_4 is not None:
                stage2(*s2_4)
                s2_4 = None
        pes5 = {}
        pps_unit(5, 0, pexp_prev, pes5)
        pps_unit(5, 1, pexp_prev, pes5)
        st5 = stage1b(5, stage1a(5, cps_last), pes5)
        stage2(5, st5)


def _prep_inputs(hidden_states, prompt_tokens, gating_factor, attention_mask,
                 Wq, bq, Wk, bk, Wv, bv):
    bf = ml_dtypes.bfloat16
    hs = np.asarray(hidden_states, np.float32)
    mask = np.asarray(attention_mask, np.float32).reshape(B, S)
    wqT = np.ascontiguousarray(np.asarray(Wq, np.float32).T).astype(bf)
    wkT = np.ascontiguousarray(np.asarray(Wk, np.float32).T).astype(bf)
    # augmented WvT: [din, 780], col 65h+j = Wv.T[:, 64h+j], col 65h+64 = 0
    wvT_f = np.asarray(Wv, np.float32).T  # [din, dout]
    wvT_aug = np.zeros((D, VW), np.float32)
    idx = np.arange(D)
    aug_cols = (idx // DH) * (DH + 1) + (idx % DH)
    wvT_aug[:, aug_cols] = wvT_f
    wvT_aug = wvT_aug.astype(bf)
    bq_c = np.asarray(bq, np.float32).reshape(D, 1)
    bk_c = np.asarray(bk, np.float32).reshape(D, 1)
    bv_aug = np.zeros(VW, np.float32)
    bv_aug[aug_cols] = np.asarray(bv, np.float32)
    bv_aug[DH::DH + 1] = 1.0
    bvaug_bc = np.ascontiguousarray(
        np.broadcast_to(bv_aug, (128, VW)), np.float32)
    pT = np.ascontiguousarray(
        np.asarray(prompt_tokens, np.float32)[0].T).astype(bf)
    gat_row = np.repeat(
        np.asarray(gating_factor, np.float32).reshape(H), DH + 1)
    gat = np.ascontiguousarray(
        np.broadcast_to(gat_row, (128, VW)), np.float32)

    shared = dict(wqT=wqT, wkT=wkT, wvT=wvT_aug, bq=bq_c, bk=bk_c,
                  bvaug=bvaug_bc, promptT=pT, gating=gat)
    in_maps = []
    for b in range(B):
        m = dict(shared)
        m["hsT"] = np.ascontiguousarray(hs[b].T).astype(bf)
        m["mask"] = np.ascontiguousarray(mask[b].reshape(S, 1))
        in_maps.append(m)
    return in_maps


def kernel(**inputs):
    global LAST_RESULTS
    if "nc" not in _CACHE:
        _CACHE["nc"] = _build_nc()
    nc = _CACHE["nc"]
    in_maps = _prep_inputs(**inputs)
    res = None
    for attempt in range(3):
        try:
            res = run_bass_kernel_spmd(nc, in_maps, list(range(B)))
            break
        except ModuleNotFoundError:
            # BASS_TRACE set but this image lacks antenv.axon_hooks
            import os

            os.environ["BASS_NEVER_TRACE"] = "1"
            if attempt == 2:
                raise
        except Exception:
            # transient NRT_EXEC_UNIT_UNRECOVERABLE on a cold device has
            # been observed; a retry on the same session recovers
            if attempt == 2:
                raise
    LAST_RESULTS = res
    out = np.empty((B, S, D), np.float32)
    for b in range(B):
        out[b] = res.results[b]["outT"].T
    return out
